# revision 8
# baseline (speedup 1.0000x reference)
"""2D Haar DWT (level 1) Trainium2 Bass kernel — fp16 I/O.

Input  x: [16, 64, 256, 256] f32
Output y: [16, 256, 128, 128] f32, y[n, s*64+c, i, j] = Haar mix s of the
2x2 block x[n, c, 2i:2i+2, 2j:2j+2].

Sharding: pure data parallel over the batch dim — core k gets batches
[2k, 2k+2).

The transform is pure data movement (out bytes == in bytes), so the kernel
is HBM-bound: per-core f32 traffic would be 67 MB (~187 us at the 358 GB/s
per-NC HBM limit). The rel-err budget (2e-2) admits fp16, halving traffic
to 33.5 MB/core. The host:
  - scales x by 0.5 (exact power of two — folds the whole Haar
    normalization, so the device does pure +/- butterflies),
  - casts to fp16,
  - de-interleaves even/odd columns to [n, c, h, 2, 128] so BOTH device
    butterfly stages are unit-stride (DVE 2x_1P perf mode needs 16-bit
    dtype + step 1 + 4B alignment; a stride-2 stage would run 1x),
  - un-scrambles the device's group-major output layout while upcasting
    fp16 -> f32 on the way out.

Measured critical path (v1-v3 traces): first_load + DVE_total + last_store,
with DMA ~saturated underneath (~406 GB/s while busy with >=2 MB transfers,
only ~348 GB/s with 1-2 MB ones — so keep transfers big in the middle).
DVE total is fixed at ~70 us (16.8M butterfly outputs at the fp16 2x rate,
241 G elem/s); GpSimd offload measured NEGATIVE (SBUF contention slowed DVE
25%). Hence:
  - groups of G channels, G in [8, 8, 16, 32 | 32, 32, 16, 8, 8, (8)]:
    small groups head and tail so the un-hideable first load (1 MB ~3 us
    vs 4 MB ~11 us) and final store (0.5 MB) are short, big groups in the
    middle for DMA efficiency,
  - per group: one contiguous load, 4 DVE butterfly ops, two contiguous
    stores (oadd block = subbands 0,1; osub block = subbands 2,3),
  - loads on the sync HWDGE ring, stores on the scalar ring, so loads
    never queue behind stores.
Layout per group (Q = 128/G partitions per channel):
  it[p=(c,q), (o t j)] = x[n, c0 + p//Q, (H/Q)*(p%Q) + o, t, j]
  output row i = q*G + r for row-pair r in [0, G).
"""

import sys

sys.path.insert(0, "/opt/trn_rl_repo")

import numpy as np

import concourse.bacc as bacc
import concourse.mybir as mybir
from concourse.tile import TileContext

N_CORES = 8
N_PER_CORE = 2  # batches per core
C = 64  # input channels
H = 256
W = 256
W2 = W // 2
F16 = mybir.dt.float16

# Per batch item: group sizes (must divide 128 and sum to C).
GROUPS = [
    [8, 8, 16, 32],  # n=0: start small so the first load is short
    [32, 16, 8, 8],  # n=1: end small so the final store is short
]
assert all(sum(gs) == C for gs in GROUPS)
# Flat fp16 output elems per batch item: 4 subbands x C x 128 x 128
Y_PER_N = 4 * C * (H // 2) * W2


def build_nc():
    nc = bacc.Bacc("TRN2", target_bir_lowering=False, debug=False)
    x = nc.dram_tensor("x", [N_PER_CORE, C, H, 2, W2], F16, kind="ExternalInput")
    # Kernel-private flat output: per (n, group): oadd block then osub block,
    # each [128, G*256] row-major. Host unscrambles during the f32 upcast.
    y = nc.dram_tensor("y", [N_PER_CORE, Y_PER_N], F16, kind="ExternalOutput")

    with TileContext(nc) as tc:
        with (
            tc.tile_pool(name="inpool", bufs=2) as inpool,
            tc.tile_pool(name="sdpool", bufs=2) as sdpool,
            tc.tile_pool(name="outpool", bufs=2) as outpool,
        ):
            for n in range(N_PER_CORE):
                c0 = 0
                off = 0
                for G in GROUPS[n]:
                    Q = 128 // G
                    # --- load: pure reshape of the contiguous group chunk
                    it = inpool.tile([128, G * 512], F16, tag="in")
                    src = x[n, c0 : c0 + G].rearrange(
                        "c (q o) t j -> (c q) (o t j)", q=Q
                    )
                    nc.sync.dma_start(out=it[:], in_=src)

                    # --- stage 1 (vertical): rows 2r / 2r+1 within a partition
                    itv = it[:].rearrange("p (r u f) -> p r u f", r=G, u=2)
                    sd = sdpool.tile([128, G * 512], F16, tag="sd")
                    sdv = sd[:].rearrange("p (v r f) -> p v r f", v=2, r=G)
                    nc.vector.tensor_add(
                        out=sdv[:, 0], in0=itv[:, :, 0], in1=itv[:, :, 1]
                    )
                    nc.vector.tensor_sub(
                        out=sdv[:, 1], in0=itv[:, :, 0], in1=itv[:, :, 1]
                    )

                    # --- stage 2 (horizontal): even/odd column planes
                    sdt = sd[:].rearrange("p (w t j) -> p w t j", t=2, j=W2)
                    oadd = outpool.tile([128, G * 256], F16, tag="oadd")
                    osub = outpool.tile([128, G * 256], F16, tag="osub")
                    oav = oadd[:].rearrange("p (w j) -> p w j", j=W2)
                    osv = osub[:].rearrange("p (w j) -> p w j", j=W2)
                    nc.vector.tensor_add(out=oav, in0=sdt[:, :, 0], in1=sdt[:, :, 1])
                    nc.vector.tensor_sub(out=osv, in0=sdt[:, :, 0], in1=sdt[:, :, 1])

                    # --- stores: two fully-contiguous DMAs into the flat y
                    blk = 128 * G * 256
                    for e, t_ in enumerate((oadd, osub)):
                        dst = y[n, off + e * blk : off + (e + 1) * blk].rearrange(
                            "(p f) -> p f", p=128
                        )
                        nc.scalar.dma_start(out=dst, in_=t_[:])
                    off += 2 * blk
                    c0 += G

    nc.finalize()
    return nc


_NC = None


def _get_nc():
    global _NC
    if _NC is None:
        _NC = build_nc()
    return _NC


def _make_in_maps(x: np.ndarray) -> list[dict]:
    """Host prep: *0.5, cast fp16, de-interleave even/odd columns."""
    x = np.asarray(x)
    assert x.shape == (16, C, H, W), x.shape
    xr = x.reshape(16, C, H, W2, 2)
    xp = np.empty((16, C, H, 2, W2), dtype=np.float16)
    np.multiply(xr[..., 0], np.float32(0.5), out=xp[:, :, :, 0, :])
    np.multiply(xr[..., 1], np.float32(0.5), out=xp[:, :, :, 1, :])
    return [
        {"x": xp[k * N_PER_CORE : (k + 1) * N_PER_CORE]} for k in range(N_CORES)
    ]


def _gather(results: list[dict]) -> np.ndarray:
    y16 = np.concatenate([r["y"] for r in results], axis=0)  # [16, Y_PER_N]
    assert y16.shape == (16, Y_PER_N), y16.shape
    out = np.empty((16, 4 * C, H // 2, W2), dtype=np.float32)
    for n_local in range(N_PER_CORE):
        c0 = 0
        off = 0
        for G in GROUPS[n_local]:
            Q = 128 // G
            blk = 128 * G * 256
            rows = slice(n_local, 16, N_PER_CORE)  # batch items with this n
            for e in range(2):
                # [16/2 sel ... ] block[b, (c q), (v r j)]
                b = y16[rows, off + e * blk : off + (e + 1) * blk]
                b = b.reshape(-1, G, Q, 2, G, W2)  # b c q v r j
                b = b.transpose(0, 3, 1, 2, 4, 5)  # b v c q r j
                b = b.reshape(-1, 2, G, H // 2, W2)
                for v in range(2):
                    s = 2 * e + v
                    out[rows, s * C + c0 : s * C + c0 + G] = b[:, v]
            off += 2 * blk
            c0 += G
    return out


def kernel(x: np.ndarray) -> np.ndarray:
    from concourse.bass_utils import run_bass_kernel_spmd

    nc = _get_nc()
    in_maps = _make_in_maps(x)
    res = run_bass_kernel_spmd(nc, in_maps, core_ids=list(range(N_CORES)))
    return _gather(res.results)


# revision 10
# speedup vs baseline: 1.0536x; 1.0536x over previous
"""2D Haar DWT (level 1) Trainium2 Bass kernel — fp16 I/O.

Input  x: [16, 64, 256, 256] f32
Output y: [16, 256, 128, 128] f32, y[n, s*64+c, i, j] = Haar mix s of the
2x2 block x[n, c, 2i:2i+2, 2j:2j+2].

Sharding: pure data parallel over the batch dim — core k gets batches
[2k, 2k+2).

The transform is pure data movement (out bytes == in bytes), so the kernel
is HBM-bound: per-core f32 traffic would be 67 MB (~187 us at the 358 GB/s
per-NC HBM limit). The rel-err budget (2e-2) admits fp16, halving traffic
to 33.5 MB/core. The host:
  - scales x by 0.5 (exact power of two — folds the whole Haar
    normalization, so the device does pure +/- butterflies),
  - casts to fp16,
  - de-interleaves even/odd columns to [n, c, h, 2, 128] so BOTH device
    butterfly stages are unit-stride (DVE 2x_1P perf mode needs 16-bit
    dtype + step 1 + 4B alignment; a stride-2 stage would run 1x),
  - un-scrambles the device's group-major output layout while upcasting
    fp16 -> f32 on the way out.

Measured structure (v1-v4 traces): span = head (~14 us: engine preamble +
DMA-queue warmup + first load) + DVE total + tail (last store). DVE is the
on-chip critical path: 16.8M butterfly outputs at the fp16 2x rate = ~70 us;
DMA (~82 us busy at 406 GB/s with >=2 MB transfers) hides under it except
head/tail. Two measured dead ends: GpSimd tensor_tensor offload slowed DVE
25% via SBUF contention; mixed tile sizes in the pools slowed ALL DVE ops
~20% (uniform [128, 16K] tiles run exactly at the 2x cycle model). Hence:
  - uniform G=32 channel groups (4 MB loads, 2 MB merged stores: oadd
    block = subbands 0,1 / osub block = subbands 2,3),
  - loads on the sync HWDGE ring, stores on the scalar ring, so loads
    never queue behind stores,
  - the LAST group's stage 2 is split per subband-pair half (4 ops, 4x
    1 MB stores) so the final store chain after the last DVE op is ~3 us
    instead of ~6.
Layout per group (Q = 128/G = 4 partitions per channel):
  it[p=(c,q), (o t j)] = x[n, c0 + p//Q, 64*(p%Q) + o, t, j]
  output row i = q*G + r for row-pair r in [0, G).
"""

import sys

sys.path.insert(0, "/opt/trn_rl_repo")

import numpy as np

import concourse.bacc as bacc
import concourse.mybir as mybir
from concourse.tile import TileContext

N_CORES = 8
N_PER_CORE = 2  # batches per core
C = 64  # input channels
H = 256
W = 256
W2 = W // 2
G = 32  # channels per group (4 MB loads)
NG = C // G  # groups per batch item
Q = 128 // G  # partitions per channel
F16 = mybir.dt.float16


def build_nc():
    nc = bacc.Bacc("TRN2", target_bir_lowering=False, debug=False)
    x = nc.dram_tensor("x", [N_PER_CORE, C, H, 2, W2], F16, kind="ExternalInput")
    # Kernel-private output layout: [n, group, tile(oadd/osub), p, f].
    # The host unscrambles this to [n, 4C, H/2, W2] during the f32 upcast.
    y = nc.dram_tensor(
        "y", [N_PER_CORE, NG, 2, 128, G * 256], F16, kind="ExternalOutput"
    )

    with TileContext(nc) as tc:
        with (
            tc.tile_pool(name="inpool", bufs=2) as inpool,
            tc.tile_pool(name="sdpool", bufs=2) as sdpool,
            tc.tile_pool(name="outpool", bufs=2) as outpool,
        ):
            for n in range(N_PER_CORE):
                for g in range(NG):
                    c0 = g * G
                    last = n == N_PER_CORE - 1 and g == NG - 1
                    # --- load: pure reshape of the 4 MB contiguous group.
                    it = inpool.tile([128, G * 512], F16, tag="in")
                    src = x[n, c0 : c0 + G].rearrange(
                        "c (q o) t j -> (c q) (o t j)", q=Q
                    )
                    nc.sync.dma_start(out=it[:], in_=src)

                    # --- stage 1 (vertical): rows 2r / 2r+1 within a partition
                    itv = it[:].rearrange("p (r u f) -> p r u f", r=G, u=2)
                    sd = sdpool.tile([128, G * 512], F16, tag="sd")
                    sdv = sd[:].rearrange("p (v r f) -> p v r f", v=2, r=G)
                    nc.vector.tensor_add(
                        out=sdv[:, 0], in0=itv[:, :, 0], in1=itv[:, :, 1]
                    )
                    nc.vector.tensor_sub(
                        out=sdv[:, 1], in0=itv[:, :, 0], in1=itv[:, :, 1]
                    )

                    # --- stage 2 (horizontal): even/odd column planes
                    sdt = sd[:].rearrange("p (w t j) -> p w t j", t=2, j=W2)
                    oadd = outpool.tile([128, G * 256], F16, tag="oadd")
                    osub = outpool.tile([128, G * 256], F16, tag="osub")
                    oav = oadd[:].rearrange("p (w j) -> p w j", j=W2)
                    osv = osub[:].rearrange("p (w j) -> p w j", j=W2)
                    half = G * 128  # elems per v-half of an output tile
                    if not last:
                        nc.vector.tensor_add(
                            out=oav, in0=sdt[:, :, 0], in1=sdt[:, :, 1]
                        )
                        nc.vector.tensor_sub(
                            out=osv, in0=sdt[:, :, 0], in1=sdt[:, :, 1]
                        )
                        nc.scalar.dma_start(out=y[n, g, 0], in_=oadd[:])
                        nc.scalar.dma_start(out=y[n, g, 1], in_=osub[:])
                    else:
                        # Tail: per-half ops + 1 MB stores so the final
                        # store chain after the last DVE op is short.
                        for v in range(2):
                            wv = slice(v * G, (v + 1) * G)
                            nc.vector.tensor_add(
                                out=oav[:, wv], in0=sdt[:, wv, 0], in1=sdt[:, wv, 1]
                            )
                            nc.vector.tensor_sub(
                                out=osv[:, wv], in0=sdt[:, wv, 0], in1=sdt[:, wv, 1]
                            )
                            # loads are done by now — the sync ring is free,
                            # so the two half-stores run on separate rings.
                            for (e, t_), eng in zip(
                                enumerate((oadd, osub)), (nc.scalar, nc.sync)
                            ):
                                eng.dma_start(
                                    out=y[n, g, e, :, v * half : (v + 1) * half],
                                    in_=t_[:, v * half : (v + 1) * half],
                                )

    nc.finalize()
    return nc


_NC = None


def _get_nc():
    global _NC
    if _NC is None:
        _NC = build_nc()
    return _NC


def _make_in_maps(x: np.ndarray) -> list[dict]:
    """Host prep: *0.5, cast fp16, de-interleave even/odd columns."""
    x = np.asarray(x)
    assert x.shape == (16, C, H, W), x.shape
    xr = x.reshape(16, C, H, W2, 2)
    xp = np.empty((16, C, H, 2, W2), dtype=np.float16)
    np.multiply(xr[..., 0], np.float32(0.5), out=xp[:, :, :, 0, :])
    np.multiply(xr[..., 1], np.float32(0.5), out=xp[:, :, :, 1, :])
    return [
        {"x": xp[k * N_PER_CORE : (k + 1) * N_PER_CORE]} for k in range(N_CORES)
    ]


def _gather(results: list[dict]) -> np.ndarray:
    y16 = np.concatenate([r["y"] for r in results], axis=0)  # [16,NG,2,128,G*256]
    # Device layout -> [n, s*C + c, i, j]:
    #   y16[n, g, e, (c q), (v r j)] ; s = 2e+v, c_full = g*G + c, i = q*G + r
    y16 = y16.reshape(16, NG, 2, G, Q, 2, G, W2)
    #                  n   g  e  c  q  v  r  j  -> n (e v) (g c) (q r) j
    y16 = y16.transpose(0, 2, 5, 1, 3, 4, 6, 7)
    return np.ascontiguousarray(y16).astype(np.float32).reshape(16, 4 * C, H // 2, W2)


def kernel(x: np.ndarray) -> np.ndarray:
    from concourse.bass_utils import run_bass_kernel_spmd

    nc = _get_nc()
    in_maps = _make_in_maps(x)
    res = run_bass_kernel_spmd(nc, in_maps, core_ids=list(range(N_CORES)))
    return _gather(res.results)
